# revision 47
# baseline (speedup 1.0000x reference)
"""HeterogeneousKANLayer forward on 8 Trainium2 NeuronCores.

Math (reference):
  xn    = tanh(x)                                  [B, I]
  base  = silu(xn)                                 [B, I]
  basis = exp(-((xn - c_j)/w)^2), c_j evenly spaced on [-1,1], w = 2/(C-1)
  out[b,o] = sum_{i,c} basis[b,i,c]*coef[i,o,c]*scale_sp[o,i]
           + sum_i base[b,i]*scale_base[o,i]

Kernel strategy (data-parallel over batch, 8 cores x 512 rows):
  Everything runs through fp8 (e4m3) DoubleRow matmuls at the PE's 2x
  fp8 rate (two 128-deep k-subtiles per instruction, ~216 ns each).
  Gaussians come from a scalar-free multiplicative ladder on DVE:
  m_0 = G_0 = exp(-((xn+1)*4.5)^2) (ACT Square+Exp), m_j = m_{j-1}*r
  with r = exp(9*xn); pure-bf16 tensor_tensor runs in the DVE 2x mode.
  m_j = G_j * exp(j^2-9j), so each fp8 cast applies s_j = exp(9j-j^2)
  via tensor_scalar/Copy-with-scale (same cost as a plain cast). Cast
  ops (fp8 out forces 1x rate; ~21 us pooled) are split between DVE
  and ACT by deadline; GpSimd is avoided entirely (slow + SBUF port
  contention stalls DVE ~3-6x). silu is one ACT op written directly
  as fp8 so the residual also runs as DoubleRow pairs (scale_base
  folded to fp8 exactly). A 64x fp8 weight scale is undone in the
  PSUM drain copy. t=0 work is quarter-granularity to prime the PE;
  t=1..3 follows in triples, consumed center-pair-major so each pair
  of casts gates only 12 matmuls. Weights are DRAM-pre-swizzled so
  every DMA is contiguous per partition. ACT stays in one table set
  (tanh/square/exp/copy) with a single switch for silu.
"""

import sys
import types

import numpy as np
import ml_dtypes

import concourse.bass as bass
import concourse.tile as tile
from concourse import bacc, mybir

N_CORES = 8
B = 4096
I = 512
O = 512
C = 10
BS = B // N_CORES          # batch rows per core (512)
NT = I // 128              # 4 i-tiles
NSP = NT * C               # 40 spline k-slots, slot s = t*10 + j
W_SPACING = 2.0 / (C - 1)
INV_W2 = 1.0 / (W_SPACING * W_SPACING)  # 20.25
A = 4.5                    # sqrt(INV_W2)
WSCALE = 64.0              # fp8 weight scale, undone at PSUM drain

_CACHE = {}


def _build():
    nc = bacc.Bacc("TRN2", target_bir_lowering=False, debug=False,
                   num_devices=N_CORES)
    f32 = mybir.dt.float32
    bf16 = mybir.dt.bfloat16
    fp8 = mybir.dt.float8e4
    AF = mybir.ActivationFunctionType
    DR = mybir.MatmulPerfMode.DoubleRow

    xt_d = nc.dram_tensor("xt", (128, NT, BS), f32, kind="ExternalInput")
    wsp_d = nc.dram_tensor("wsp", (128, NSP, O), fp8, kind="ExternalInput")
    wres_d = nc.dram_tensor("wres", (128, NT, O), fp8, kind="ExternalInput")
    out_d = nc.dram_tensor("out", (128, NT, O), f32, kind="ExternalOutput")

    with tile.TileContext(nc) as tc:
        with (
            tc.tile_pool(name="big", bufs=1) as big,
            tc.tile_pool(name="psum", bufs=1, space="PSUM") as psum,
        ):
            # ---- warm-ups: ACT table (set 0: tanh/exp/square/copy) + PE
            # p-state ramp, all off the DMA-wait critical path ----
            warm = big.tile([128, 8], f32, tag="warm")
            nc.vector.memset(warm[:], 0.0)
            bias45 = big.tile([128, 1], f32, tag="bias45")
            nc.vector.memset(bias45[:], A)
            nc.scalar.activation(out=warm[:], in_=warm[:], func=AF.Tanh)
            nc.scalar.activation(out=warm[:], in_=warm[:], func=AF.Exp)

            warm_w = big.tile([128, 512], bf16, tag="warmw")
            nc.vector.memset(warm_w[:], 0.0)
            ps_warm = psum.tile([128, 512], f32, name="pswarm")
            for _ in range(14):
                nc.tensor.matmul(ps_warm[:], warm_w[:, :128], warm_w[:],
                                 start=True, stop=True)

            # ---- DMAs: x first (heads the critical path), then weights ----
            xt_sb = big.tile([128, NT, BS], f32, tag="xt")
            nc.sync.dma_start(out=xt_sb[:, 0, :], in_=xt_d[:, 0, :])
            nc.sync.dma_start(out=xt_sb[:, 1:, :], in_=xt_d[:, 1:, :])
            wsp_a = big.tile([128, 10, O], fp8, tag="wspa")
            nc.sync.dma_start(out=wsp_a[:], in_=wsp_d[:, 0:10, :])
            wsp_b = big.tile([128, 30, O], fp8, tag="wspb")
            nc.sync.dma_start(out=wsp_b[:, 0:10, :], in_=wsp_d[:, 10:20, :])
            nc.sync.dma_start(out=wsp_b[:, 10:30, :], in_=wsp_d[:, 20:40, :])

            def wsp_slot(s):
                return (wsp_a, s) if s < 10 else (wsp_b, s - 10)
            wres_sb = big.tile([128, NT, O], fp8, tag="wres")
            nc.sync.dma_start(out=wres_sb[:], in_=wres_d[:, :, :])

            # ---- head: tanh / anchor gaussian / ladder ratio on ACT,
            # t=0 first (quarter ops) then t=1..3 in one go ----
            xn = big.tile([128, NT, BS], f32, tag="xn")
            z0 = big.tile([128, NT, BS], f32, tag="z0")
            G = [big.tile([128, NT, BS], bf16, tag=f"G{j}", name=f"G{j}")
                 for j in range(C)]
            r = big.tile([128, NT, BS], bf16, tag="r")

            def head(tsl):
                nc.scalar.activation(out=xn[:, tsl, :], in_=xt_sb[:, tsl, :],
                                     func=AF.Tanh)
                nc.scalar.activation(out=z0[:, tsl, :], in_=xn[:, tsl, :],
                                     func=AF.Square, scale=A,
                                     bias=bias45[:, 0:1])
                nc.scalar.activation(out=G[0][:, tsl, :], in_=z0[:, tsl, :],
                                     func=AF.Exp, scale=-1.0)
                nc.scalar.activation(out=r[:, tsl, :], in_=xn[:, tsl, :],
                                     func=AF.Exp, scale=2.0 * W_SPACING * INV_W2)
            # ---- gaussian ladder: m_j = m_{j-1} * r, pure bf16
            # tensor_tensor on DVE; per-center scale rides on the fp8 cast.
            # m_j = G_j * exp(j^2 - 9j), so cast scalar s_j = exp(9j - j^2).
            # s_0 = s_9 = 1 exactly, so those casts are pure copies and go
            # to the otherwise-idle GpSimd (whose tensor_scalar is slow but
            # tensor_copy is usable). slot(t, j) = t*10 + j; center j is
            # slots j, 10+j, 20+j, 30+j. All t=0 work first (it gates the
            # first matmul block), then t=1..3.
            basis = big.tile([128, NSP, BS], fp8, tag="basis")
            T0, T123 = slice(0, 1), slice(1, 4)

            def cast(e, j, tsl, gsl):
                s_j = float(np.exp(9.0 * j - j * j))
                if e == "g":
                    assert abs(s_j - 1.0) < 1e-12
                    nc.gpsimd.tensor_copy(out=basis[:, tsl, :],
                                          in_=G[j][:, gsl, :])
                elif e == "s":
                    nc.scalar.mul(out=basis[:, tsl, :], in_=G[j][:, gsl, :],
                                  mul=s_j)
                else:
                    nc.vector.tensor_scalar_mul(basis[:, tsl, :],
                                                G[j][:, gsl, :], s_j)

            # ACT queue: t0 head, cast j0-t0, t123 head, casts j7,j8-t0,
            # then the t123 casts j=1..7 (silu wedged after j=3).
            # DVE queue: both ladders + t0 casts j=1..6 + t123 j=8,9.
            # GpSimd: the two pure copies with slack (j9-t0, j0-t123).
            head(T0)
            cast("s", 0, slice(0, 1), T0)
            head(T123)

            for j in range(1, C):
                nc.vector.tensor_mul(out=G[j][:, T0, :],
                                     in0=G[j - 1][:, T0, :], in1=r[:, T0, :])
                if j <= 6 or j == 9:
                    cast("v", j, slice(j, j + 1), T0)
            cast("s", 7, slice(7, 8), T0)
            cast("s", 8, slice(8, 9), T0)
            cast("s", 0, slice(10, 40, 10), T123)

            silu2 = big.tile([128, NT, BS], fp8, tag="silu2")
            for j in range(1, C):
                nc.vector.tensor_mul(out=G[j][:, T123, :],
                                     in0=G[j - 1][:, T123, :],
                                     in1=r[:, T123, :])
                if j <= 4:
                    cast("s", j, slice(10 + j, 40, 10), T123)
                elif j == 5:
                    cast("v", j, slice(10 + j, 40, 10), T123)
                if j == 4:
                    # silu on ACT mid-stream (resid matmuls run ~then);
                    # one table-set switch, later Copy casts stay in-set
                    nc.scalar.activation(out=silu2[:], in_=xn[:],
                                         func=AF.Silu)
            cast("s", 6, slice(16, 40, 10), T123)
            for j in range(7, C):
                cast("v", j, slice(10 + j, 40, 10), T123)

            # ---- spline matmuls: fp8 DoubleRow. t=0 block first (fed by
            # the quarter-granularity phase A), then the t=1..3 blocks
            # center-pair-major so each m-level's casts gate only 12
            # matmuls. Residual (bf16) slots in after the m=1 level, where
            # the PE would otherwise wait on mid-chain casts. ----
            ps = [psum.tile([128, O], f32, name=f"ps{bt}") for bt in range(4)]

            def spline_mm(t, m, bt, start, stop=False):
                s = t * C + 2 * m
                w, ws = wsp_slot(s)
                nc.tensor.matmul(
                    ps[bt], basis[:, s:s + 2, bt * 128:(bt + 1) * 128],
                    w[:, ws:ws + 2, :], start=start, stop=stop,
                    perf_mode=DR, skip_group_check=True)

            for m in range(C // 2):
                if m == 4:
                    # keep the PE hot while waiting on the late t0 casts
                    for _ in range(3):
                        nc.tensor.matmul(ps_warm[:], warm_w[:, :128],
                                         warm_w[:], start=True, stop=True)
                for bt in range(4):
                    spline_mm(0, m, bt, start=(m == 0))
            # keep the PE busy/hot through the t0->t123 cast wait
            for _ in range(6):
                nc.tensor.matmul(ps_warm[:], warm_w[:, :128], warm_w[:],
                                 start=True, stop=True)
            for m in range(C // 2 - 1):
                for t in (1, 2, 3):
                    for bt in range(4):
                        spline_mm(t, m, bt, start=False)
                if m == 2:
                    for bt in range(4):
                        for t in (0, 2):
                            nc.tensor.matmul(
                                ps[bt],
                                silu2[:, t:t + 2, bt * 128:(bt + 1) * 128],
                                wres_sb[:, t:t + 2, :], start=False,
                                stop=False, perf_mode=DR,
                                skip_group_check=True)
            # last m-level bank-major with staggered drain + out DMA
            out_sb = big.tile([128, NT, O], f32, tag="out")
            for bt in range(4):
                for t in (1, 2, 3):
                    spline_mm(t, C // 2 - 1, bt, start=False, stop=(t == 3))
                if bt == 3:
                    # split the last bank so its out-DMA starts sooner
                    nc.scalar.mul(out=out_sb[:, bt, 0:256],
                                  in_=ps[bt][:, 0:256], mul=1.0 / WSCALE)
                    nc.sync.dma_start(out=out_d[:, bt, 0:256],
                                      in_=out_sb[:, bt, 0:256])
                    nc.vector.tensor_scalar_mul(out_sb[:, bt, 256:512],
                                                ps[bt][:, 256:512],
                                                1.0 / WSCALE)
                    nc.sync.dma_start(out=out_d[:, bt, 256:512],
                                      in_=out_sb[:, bt, 256:512])
                    continue
                if bt % 2 == 0:
                    nc.scalar.mul(out=out_sb[:, bt, :], in_=ps[bt][:],
                                  mul=1.0 / WSCALE)
                else:
                    nc.vector.tensor_scalar_mul(out_sb[:, bt, :], ps[bt][:],
                                                1.0 / WSCALE)
                nc.sync.dma_start(out=out_d[:, bt, :], in_=out_sb[:, bt, :])
    nc.finalize()
    return nc


def _prep_inputs(x, coef, scale_base, scale_sp):
    """Host-side shard + layout prep (cheap numpy reshapes/casts)."""
    x = np.asarray(x, dtype=np.float32)
    coef = np.asarray(coef, dtype=np.float32)
    scale_base = np.asarray(scale_base, dtype=np.float32)
    scale_sp = np.asarray(scale_sp, dtype=np.float32)

    # spline weights: slot s = t*10 + j -> wsp[p, s, o] = wf[t*128+p, o, j]
    wf = coef * scale_sp.T[:, :, None] * WSCALE           # [I, O, C]
    wsp = wf.reshape(NT, 128, O, C).transpose(1, 0, 3, 2)  # [128, NT, C, O]
    wsp = np.ascontiguousarray(wsp.reshape(128, NSP, O)).astype(
        ml_dtypes.float8_e4m3)
    # residual weights carry the fp8 WSCALE (undone at PSUM drain)
    wres = (scale_base.T * WSCALE).reshape(NT, 128, O)
    wres = np.ascontiguousarray(wres.transpose(1, 0, 2)).astype(
        ml_dtypes.float8_e4m3)

    in_maps = []
    for k in range(N_CORES):
        xs = x[k * BS:(k + 1) * BS, :]                     # [BS, I]
        xt = np.ascontiguousarray(
            xs.T.reshape(NT, 128, BS).transpose(1, 0, 2))  # [128, NT, BS]
        in_maps.append({"xt": xt, "wsp": wsp, "wres": wres})
    return in_maps


def _run(in_maps, trace=False):
    if "antenv.axon_hooks" not in sys.modules:
        try:
            from trn_agent_boot.trn_boot import _ntff_profile_via_ctypes
            _hook = _ntff_profile_via_ctypes("/opt/axon/libaxon_pjrt.so")
            _mod = types.ModuleType("antenv.axon_hooks")
            _mod.get_axon_ntff_profile_hook = lambda: _hook
            sys.modules["antenv.axon_hooks"] = _mod
        except Exception:
            pass
    from concourse.bass_utils import run_bass_kernel_spmd

    if "nc" not in _CACHE:
        _CACHE["nc"] = _build()
    return run_bass_kernel_spmd(_CACHE["nc"], in_maps,
                                core_ids=list(range(N_CORES)), trace=trace)


def kernel(x, coef, scale_base, scale_sp):
    in_maps = _prep_inputs(x, coef, scale_base, scale_sp)
    res = _run(in_maps, trace=False)
    out = np.concatenate(
        [res.results[k]["out"].transpose(1, 0, 2).reshape(BS, O)
         for k in range(N_CORES)], axis=0)
    return out.astype(np.float32)


# revision 48
# speedup vs baseline: 1.0036x; 1.0036x over previous
"""HeterogeneousKANLayer forward on 8 Trainium2 NeuronCores.

Math (reference):
  xn    = tanh(x)                                  [B, I]
  base  = silu(xn)                                 [B, I]
  basis = exp(-((xn - c_j)/w)^2), c_j evenly spaced on [-1,1], w = 2/(C-1)
  out[b,o] = sum_{i,c} basis[b,i,c]*coef[i,o,c]*scale_sp[o,i]
           + sum_i base[b,i]*scale_base[o,i]

Kernel strategy (data-parallel over batch, 8 cores x 512 rows):
  Everything runs through fp8 (e4m3) DoubleRow matmuls at the PE's 2x
  fp8 rate (two 128-deep k-subtiles per instruction, ~216 ns each).
  Gaussians come from a scalar-free multiplicative ladder on DVE:
  m_0 = G_0 = exp(-((xn+1)*4.5)^2) (ACT Square+Exp), m_j = m_{j-1}*r
  with r = exp(9*xn); pure-bf16 tensor_tensor runs in the DVE 2x mode.
  m_j = G_j * exp(j^2-9j), so each fp8 cast applies s_j = exp(9j-j^2)
  via tensor_scalar/Copy-with-scale (same cost as a plain cast). Cast
  ops (fp8 out forces 1x rate; ~21 us pooled) are split between DVE
  and ACT by deadline; GpSimd is avoided entirely (slow + SBUF port
  contention stalls DVE ~3-6x). silu is one ACT op written directly
  as fp8 so the residual also runs as DoubleRow pairs (scale_base
  folded to fp8 exactly). A 64x fp8 weight scale is undone in the
  PSUM drain copy. t=0 work is quarter-granularity to prime the PE;
  t=1..3 follows in triples, consumed center-pair-major so each pair
  of casts gates only 12 matmuls. Weights are DRAM-pre-swizzled so
  every DMA is contiguous per partition. ACT stays in one table set
  (tanh/square/exp/copy) with a single switch for silu.
"""

import sys
import types

import numpy as np
import ml_dtypes

import concourse.bass as bass
import concourse.tile as tile
from concourse import bacc, mybir

N_CORES = 8
B = 4096
I = 512
O = 512
C = 10
BS = B // N_CORES          # batch rows per core (512)
NT = I // 128              # 4 i-tiles
NSP = NT * C               # 40 spline k-slots, slot s = t*10 + j
W_SPACING = 2.0 / (C - 1)
INV_W2 = 1.0 / (W_SPACING * W_SPACING)  # 20.25
A = 4.5                    # sqrt(INV_W2)
WSCALE = 64.0              # fp8 weight scale, undone at PSUM drain

_CACHE = {}


def _build():
    nc = bacc.Bacc("TRN2", target_bir_lowering=False, debug=False,
                   num_devices=N_CORES)
    f32 = mybir.dt.float32
    bf16 = mybir.dt.bfloat16
    fp8 = mybir.dt.float8e4
    AF = mybir.ActivationFunctionType
    DR = mybir.MatmulPerfMode.DoubleRow

    xt_d = nc.dram_tensor("xt", (128, NT, BS), f32, kind="ExternalInput")
    wsp_d = nc.dram_tensor("wsp", (128, NSP, O), fp8, kind="ExternalInput")
    wres_d = nc.dram_tensor("wres", (128, NT, O), fp8, kind="ExternalInput")
    out_d = nc.dram_tensor("out", (128, NT, O), f32, kind="ExternalOutput")

    with tile.TileContext(nc) as tc:
        with (
            tc.tile_pool(name="big", bufs=1) as big,
            tc.tile_pool(name="psum", bufs=1, space="PSUM") as psum,
        ):
            # ---- warm-ups: ACT table (set 0: tanh/exp/square/copy) + PE
            # p-state ramp, all off the DMA-wait critical path ----
            warm = big.tile([128, 8], f32, tag="warm")
            nc.vector.memset(warm[:], 0.0)
            bias45 = big.tile([128, 1], f32, tag="bias45")
            nc.vector.memset(bias45[:], A)
            nc.scalar.activation(out=warm[:], in_=warm[:], func=AF.Tanh)
            nc.scalar.activation(out=warm[:], in_=warm[:], func=AF.Exp)

            warm_w = big.tile([128, 512], bf16, tag="warmw")
            nc.vector.memset(warm_w[:], 0.0)
            ps_warm = psum.tile([128, 512], f32, name="pswarm")
            for _ in range(14):
                nc.tensor.matmul(ps_warm[:], warm_w[:, :128], warm_w[:],
                                 start=True, stop=True)

            # ---- DMAs: x first (heads the critical path), then weights ----
            xt_sb = big.tile([128, NT, BS], f32, tag="xt")
            nc.sync.dma_start(out=xt_sb[:, 0, :], in_=xt_d[:, 0, :])
            nc.sync.dma_start(out=xt_sb[:, 1:, :], in_=xt_d[:, 1:, :])
            wsp_a = big.tile([128, 10, O], fp8, tag="wspa")
            nc.sync.dma_start(out=wsp_a[:], in_=wsp_d[:, 0:10, :])
            wsp_b = big.tile([128, 30, O], fp8, tag="wspb")
            nc.sync.dma_start(out=wsp_b[:, 0:10, :], in_=wsp_d[:, 10:20, :])
            nc.sync.dma_start(out=wsp_b[:, 10:30, :], in_=wsp_d[:, 20:40, :])

            def wsp_slot(s):
                return (wsp_a, s) if s < 10 else (wsp_b, s - 10)
            wres_sb = big.tile([128, NT, O], fp8, tag="wres")
            nc.sync.dma_start(out=wres_sb[:], in_=wres_d[:, :, :])

            # ---- head: tanh / anchor gaussian / ladder ratio on ACT,
            # t=0 first (quarter ops) then t=1..3 in one go ----
            xn = big.tile([128, NT, BS], f32, tag="xn")
            z0 = big.tile([128, NT, BS], f32, tag="z0")
            G = [big.tile([128, NT, BS], bf16, tag=f"G{j}", name=f"G{j}")
                 for j in range(C)]
            r = big.tile([128, NT, BS], bf16, tag="r")

            def head(tsl):
                nc.scalar.activation(out=xn[:, tsl, :], in_=xt_sb[:, tsl, :],
                                     func=AF.Tanh)
                nc.scalar.activation(out=z0[:, tsl, :], in_=xn[:, tsl, :],
                                     func=AF.Square, scale=A,
                                     bias=bias45[:, 0:1])
                nc.scalar.activation(out=G[0][:, tsl, :], in_=z0[:, tsl, :],
                                     func=AF.Exp, scale=-1.0)
                nc.scalar.activation(out=r[:, tsl, :], in_=xn[:, tsl, :],
                                     func=AF.Exp, scale=2.0 * W_SPACING * INV_W2)
            # ---- gaussian ladder: m_j = m_{j-1} * r, pure bf16
            # tensor_tensor on DVE; per-center scale rides on the fp8 cast.
            # m_j = G_j * exp(j^2 - 9j), so cast scalar s_j = exp(9j - j^2).
            # s_0 = s_9 = 1 exactly, so those casts are pure copies and go
            # to the otherwise-idle GpSimd (whose tensor_scalar is slow but
            # tensor_copy is usable). slot(t, j) = t*10 + j; center j is
            # slots j, 10+j, 20+j, 30+j. All t=0 work first (it gates the
            # first matmul block), then t=1..3.
            basis = big.tile([128, NSP, BS], fp8, tag="basis")
            T0, T123 = slice(0, 1), slice(1, 4)

            def cast(e, j, tsl, gsl):
                s_j = float(np.exp(9.0 * j - j * j))
                if e == "g":
                    assert abs(s_j - 1.0) < 1e-12
                    nc.gpsimd.tensor_copy(out=basis[:, tsl, :],
                                          in_=G[j][:, gsl, :])
                elif e == "s":
                    nc.scalar.mul(out=basis[:, tsl, :], in_=G[j][:, gsl, :],
                                  mul=s_j)
                else:
                    nc.vector.tensor_scalar_mul(basis[:, tsl, :],
                                                G[j][:, gsl, :], s_j)

            # ACT queue: t0 head, cast j0-t0, t123 head, casts j7,j8-t0,
            # then the t123 casts j=1..7 (silu wedged after j=3).
            # DVE queue: both ladders + t0 casts j=1..6 + t123 j=8,9.
            # GpSimd: the two pure copies with slack (j9-t0, j0-t123).
            head(T0)
            cast("s", 0, slice(0, 1), T0)
            head(T123)

            for j in range(1, C):
                nc.vector.tensor_mul(out=G[j][:, T0, :],
                                     in0=G[j - 1][:, T0, :], in1=r[:, T0, :])
                if j <= 6 or j == 9:
                    cast("v", j, slice(j, j + 1), T0)
            cast("s", 7, slice(7, 8), T0)
            cast("s", 8, slice(8, 9), T0)
            cast("s", 0, slice(10, 40, 10), T123)

            silu2 = big.tile([128, NT, BS], fp8, tag="silu2")
            for j in range(1, C):
                nc.vector.tensor_mul(out=G[j][:, T123, :],
                                     in0=G[j - 1][:, T123, :],
                                     in1=r[:, T123, :])
                if j <= 4:
                    cast("s", j, slice(10 + j, 40, 10), T123)
                elif j == 5:
                    cast("v", j, slice(10 + j, 40, 10), T123)
                if j == 4:
                    # silu on ACT mid-stream (resid matmuls run ~then);
                    # one table-set switch, later Copy casts stay in-set
                    nc.scalar.activation(out=silu2[:], in_=xn[:],
                                         func=AF.Silu)
            cast("s", 6, slice(16, 40, 10), T123)
            for j in range(7, C):
                cast("v", j, slice(10 + j, 40, 10), T123)

            # ---- spline matmuls: fp8 DoubleRow. t=0 block first (fed by
            # the quarter-granularity phase A), then the t=1..3 blocks
            # center-pair-major so each m-level's casts gate only 12
            # matmuls. Residual (bf16) slots in after the m=1 level, where
            # the PE would otherwise wait on mid-chain casts. ----
            ps = [psum.tile([128, O], f32, name=f"ps{bt}") for bt in range(4)]

            def spline_mm(t, m, bt, start, stop=False):
                s = t * C + 2 * m
                w, ws = wsp_slot(s)
                nc.tensor.matmul(
                    ps[bt], basis[:, s:s + 2, bt * 128:(bt + 1) * 128],
                    w[:, ws:ws + 2, :], start=start, stop=stop,
                    perf_mode=DR, skip_group_check=True)

            for m in range(C // 2):
                for bt in range(4):
                    spline_mm(0, m, bt, start=(m == 0))
            # keep the PE busy/hot through the t0->t123 cast wait
            for _ in range(6):
                nc.tensor.matmul(ps_warm[:], warm_w[:, :128], warm_w[:],
                                 start=True, stop=True)
            for m in range(C // 2 - 1):
                for t in (1, 2, 3):
                    for bt in range(4):
                        spline_mm(t, m, bt, start=False)
                if m == 2:
                    for bt in range(4):
                        for t in (0, 2):
                            nc.tensor.matmul(
                                ps[bt],
                                silu2[:, t:t + 2, bt * 128:(bt + 1) * 128],
                                wres_sb[:, t:t + 2, :], start=False,
                                stop=False, perf_mode=DR,
                                skip_group_check=True)
            # last m-level bank-major with staggered drain + out DMA
            out_sb = big.tile([128, NT, O], f32, tag="out")
            for bt in range(4):
                for t in (1, 2, 3):
                    spline_mm(t, C // 2 - 1, bt, start=False, stop=(t == 3))
                if bt == 3:
                    # split the last bank so its out-DMA starts sooner
                    nc.scalar.mul(out=out_sb[:, bt, 0:256],
                                  in_=ps[bt][:, 0:256], mul=1.0 / WSCALE)
                    nc.sync.dma_start(out=out_d[:, bt, 0:256],
                                      in_=out_sb[:, bt, 0:256])
                    nc.vector.tensor_scalar_mul(out_sb[:, bt, 256:512],
                                                ps[bt][:, 256:512],
                                                1.0 / WSCALE)
                    nc.sync.dma_start(out=out_d[:, bt, 256:512],
                                      in_=out_sb[:, bt, 256:512])
                    continue
                if bt % 2 == 0:
                    nc.scalar.mul(out=out_sb[:, bt, :], in_=ps[bt][:],
                                  mul=1.0 / WSCALE)
                else:
                    nc.vector.tensor_scalar_mul(out_sb[:, bt, :], ps[bt][:],
                                                1.0 / WSCALE)
                nc.sync.dma_start(out=out_d[:, bt, :], in_=out_sb[:, bt, :])
    nc.finalize()
    return nc


def _prep_inputs(x, coef, scale_base, scale_sp):
    """Host-side shard + layout prep (cheap numpy reshapes/casts)."""
    x = np.asarray(x, dtype=np.float32)
    coef = np.asarray(coef, dtype=np.float32)
    scale_base = np.asarray(scale_base, dtype=np.float32)
    scale_sp = np.asarray(scale_sp, dtype=np.float32)

    # spline weights: slot s = t*10 + j -> wsp[p, s, o] = wf[t*128+p, o, j]
    wf = coef * scale_sp.T[:, :, None] * WSCALE           # [I, O, C]
    wsp = wf.reshape(NT, 128, O, C).transpose(1, 0, 3, 2)  # [128, NT, C, O]
    wsp = np.ascontiguousarray(wsp.reshape(128, NSP, O)).astype(
        ml_dtypes.float8_e4m3)
    # residual weights carry the fp8 WSCALE (undone at PSUM drain)
    wres = (scale_base.T * WSCALE).reshape(NT, 128, O)
    wres = np.ascontiguousarray(wres.transpose(1, 0, 2)).astype(
        ml_dtypes.float8_e4m3)

    in_maps = []
    for k in range(N_CORES):
        xs = x[k * BS:(k + 1) * BS, :]                     # [BS, I]
        xt = np.ascontiguousarray(
            xs.T.reshape(NT, 128, BS).transpose(1, 0, 2))  # [128, NT, BS]
        in_maps.append({"xt": xt, "wsp": wsp, "wres": wres})
    return in_maps


def _run(in_maps, trace=False):
    if "antenv.axon_hooks" not in sys.modules:
        try:
            from trn_agent_boot.trn_boot import _ntff_profile_via_ctypes
            _hook = _ntff_profile_via_ctypes("/opt/axon/libaxon_pjrt.so")
            _mod = types.ModuleType("antenv.axon_hooks")
            _mod.get_axon_ntff_profile_hook = lambda: _hook
            sys.modules["antenv.axon_hooks"] = _mod
        except Exception:
            pass
    from concourse.bass_utils import run_bass_kernel_spmd

    if "nc" not in _CACHE:
        _CACHE["nc"] = _build()
    return run_bass_kernel_spmd(_CACHE["nc"], in_maps,
                                core_ids=list(range(N_CORES)), trace=trace)


def kernel(x, coef, scale_base, scale_sp):
    in_maps = _prep_inputs(x, coef, scale_base, scale_sp)
    res = _run(in_maps, trace=False)
    out = np.concatenate(
        [res.results[k]["out"].transpose(1, 0, 2).reshape(BS, O)
         for k in range(N_CORES)], axis=0)
    return out.astype(np.float32)


# revision 49
# speedup vs baseline: 1.0252x; 1.0214x over previous
"""HeterogeneousKANLayer forward on 8 Trainium2 NeuronCores.

Math (reference):
  xn    = tanh(x)                                  [B, I]
  base  = silu(xn)                                 [B, I]
  basis = exp(-((xn - c_j)/w)^2), c_j evenly spaced on [-1,1], w = 2/(C-1)
  out[b,o] = sum_{i,c} basis[b,i,c]*coef[i,o,c]*scale_sp[o,i]
           + sum_i base[b,i]*scale_base[o,i]

Kernel strategy (data-parallel over batch, 8 cores x 512 rows):
  Everything runs through fp8 (e4m3) DoubleRow matmuls at the PE's 2x
  fp8 rate (two 128-deep k-subtiles per instruction, ~216 ns each).
  Gaussians come from a scalar-free multiplicative ladder on DVE:
  m_0 = G_0 = exp(-((xn+1)*4.5)^2) (ACT Square+Exp), m_j = m_{j-1}*r
  with r = exp(9*xn); pure-bf16 tensor_tensor runs in the DVE 2x mode.
  m_j = G_j * exp(j^2-9j), so each fp8 cast applies s_j = exp(9j-j^2)
  via tensor_scalar/Copy-with-scale (same cost as a plain cast). Cast
  ops (fp8 out forces 1x rate; ~21 us pooled) are split between DVE
  and ACT by deadline; GpSimd is avoided entirely (slow + SBUF port
  contention stalls DVE ~3-6x). silu is one ACT op written directly
  as fp8 so the residual also runs as DoubleRow pairs (scale_base
  folded to fp8 exactly). A 64x fp8 weight scale is undone in the
  PSUM drain copy. t=0 work is quarter-granularity to prime the PE;
  t=1..3 follows in triples, consumed center-pair-major so each pair
  of casts gates only 12 matmuls. Weights are DRAM-pre-swizzled so
  every DMA is contiguous per partition. ACT stays in one table set
  (tanh/square/exp/copy) with a single switch for silu.
"""

import sys
import types

import numpy as np
import ml_dtypes

import concourse.bass as bass
import concourse.tile as tile
from concourse import bacc, mybir

N_CORES = 8
B = 4096
I = 512
O = 512
C = 10
BS = B // N_CORES          # batch rows per core (512)
NT = I // 128              # 4 i-tiles
NSP = NT * C               # 40 spline k-slots, slot s = t*10 + j
W_SPACING = 2.0 / (C - 1)
INV_W2 = 1.0 / (W_SPACING * W_SPACING)  # 20.25
A = 4.5                    # sqrt(INV_W2)
WSCALE = 64.0              # fp8 weight scale, undone at PSUM drain

_CACHE = {}


def _build():
    nc = bacc.Bacc("TRN2", target_bir_lowering=False, debug=False,
                   num_devices=N_CORES)
    f32 = mybir.dt.float32
    bf16 = mybir.dt.bfloat16
    fp8 = mybir.dt.float8e4
    AF = mybir.ActivationFunctionType
    DR = mybir.MatmulPerfMode.DoubleRow

    xt_d = nc.dram_tensor("xt", (128, NT, BS), f32, kind="ExternalInput")
    wsp_d = nc.dram_tensor("wsp", (128, NSP, O), fp8, kind="ExternalInput")
    wres_d = nc.dram_tensor("wres", (128, NT, O), fp8, kind="ExternalInput")
    out_d = nc.dram_tensor("out", (128, NT, O), f32, kind="ExternalOutput")

    with tile.TileContext(nc) as tc:
        with (
            tc.tile_pool(name="big", bufs=1) as big,
            tc.tile_pool(name="psum", bufs=1, space="PSUM") as psum,
        ):
            # ---- warm-ups: ACT table (set 0: tanh/exp/square/copy) + PE
            # p-state ramp, all off the DMA-wait critical path ----
            warm = big.tile([128, 8], f32, tag="warm")
            nc.vector.memset(warm[:], 0.0)
            bias45 = big.tile([128, 1], f32, tag="bias45")
            nc.vector.memset(bias45[:], A)
            nc.scalar.activation(out=warm[:], in_=warm[:], func=AF.Tanh)
            nc.scalar.activation(out=warm[:], in_=warm[:], func=AF.Exp)

            warm_w = big.tile([128, 512], bf16, tag="warmw")
            nc.vector.memset(warm_w[:], 0.0)
            ps_warm = psum.tile([128, 512], f32, name="pswarm")
            for _ in range(14):
                nc.tensor.matmul(ps_warm[:], warm_w[:, :128], warm_w[:],
                                 start=True, stop=True)

            # ---- DMAs: x first (heads the critical path), then weights ----
            xt_sb = big.tile([128, NT, BS], f32, tag="xt")
            nc.sync.dma_start(out=xt_sb[:, 0, :], in_=xt_d[:, 0, :])
            nc.sync.dma_start(out=xt_sb[:, 1:, :], in_=xt_d[:, 1:, :])
            wsp_a = big.tile([128, 10, O], fp8, tag="wspa")
            nc.sync.dma_start(out=wsp_a[:], in_=wsp_d[:, 0:10, :])
            wsp_b = big.tile([128, 30, O], fp8, tag="wspb")
            nc.sync.dma_start(out=wsp_b[:, 0:10, :], in_=wsp_d[:, 10:20, :])
            nc.sync.dma_start(out=wsp_b[:, 10:30, :], in_=wsp_d[:, 20:40, :])

            def wsp_slot(s):
                return (wsp_a, s) if s < 10 else (wsp_b, s - 10)
            wres_sb = big.tile([128, NT, O], fp8, tag="wres")
            nc.sync.dma_start(out=wres_sb[:], in_=wres_d[:, :, :])

            # ---- head: tanh / anchor gaussian / ladder ratio on ACT,
            # t=0 first (quarter ops) then t=1..3 in one go ----
            xn = big.tile([128, NT, BS], f32, tag="xn")
            z0 = big.tile([128, NT, BS], f32, tag="z0")
            G = [big.tile([128, NT, BS], bf16, tag=f"G{j}", name=f"G{j}")
                 for j in range(C)]
            r = big.tile([128, NT, BS], bf16, tag="r")

            def head(tsl):
                nc.scalar.activation(out=xn[:, tsl, :], in_=xt_sb[:, tsl, :],
                                     func=AF.Tanh)
                nc.scalar.activation(out=z0[:, tsl, :], in_=xn[:, tsl, :],
                                     func=AF.Square, scale=A,
                                     bias=bias45[:, 0:1])
                nc.scalar.activation(out=G[0][:, tsl, :], in_=z0[:, tsl, :],
                                     func=AF.Exp, scale=-1.0)
                nc.scalar.activation(out=r[:, tsl, :], in_=xn[:, tsl, :],
                                     func=AF.Exp, scale=2.0 * W_SPACING * INV_W2)
            # ---- gaussian ladder: m_j = m_{j-1} * r, pure bf16
            # tensor_tensor on DVE; per-center scale rides on the fp8 cast.
            # m_j = G_j * exp(j^2 - 9j), so cast scalar s_j = exp(9j - j^2).
            # s_0 = s_9 = 1 exactly, so those casts are pure copies and go
            # to the otherwise-idle GpSimd (whose tensor_scalar is slow but
            # tensor_copy is usable). slot(t, j) = t*10 + j; center j is
            # slots j, 10+j, 20+j, 30+j. All t=0 work first (it gates the
            # first matmul block), then t=1..3.
            basis = big.tile([128, NSP, BS], fp8, tag="basis")
            T0, T123 = slice(0, 1), slice(1, 4)

            def cast(e, j, tsl, gsl):
                s_j = float(np.exp(9.0 * j - j * j))
                if e == "g":
                    assert abs(s_j - 1.0) < 1e-12
                    nc.gpsimd.tensor_copy(out=basis[:, tsl, :],
                                          in_=G[j][:, gsl, :])
                elif e == "s":
                    nc.scalar.mul(out=basis[:, tsl, :], in_=G[j][:, gsl, :],
                                  mul=s_j)
                else:
                    nc.vector.tensor_scalar_mul(basis[:, tsl, :],
                                                G[j][:, gsl, :], s_j)

            # ACT queue: t0 head, cast j0-t0, t123 head, casts j7,j8-t0,
            # then the t123 casts j=1..7 (silu wedged after j=3).
            # DVE queue: both ladders + t0 casts j=1..6 + t123 j=8,9.
            # GpSimd: the two pure copies with slack (j9-t0, j0-t123).
            head(T0)
            cast("s", 0, slice(0, 1), T0)
            head(T123)

            for j in range(1, C):
                nc.vector.tensor_mul(out=G[j][:, T0, :],
                                     in0=G[j - 1][:, T0, :], in1=r[:, T0, :])
                if j <= 6 or j == 9:
                    cast("v", j, slice(j, j + 1), T0)
            cast("s", 7, slice(7, 8), T0)
            cast("s", 8, slice(8, 9), T0)
            cast("s", 0, slice(10, 40, 10), T123)

            silu2 = big.tile([128, NT, BS], fp8, tag="silu2")
            for j in range(1, C):
                nc.vector.tensor_mul(out=G[j][:, T123, :],
                                     in0=G[j - 1][:, T123, :],
                                     in1=r[:, T123, :])
                if j <= 4:
                    cast("s", j, slice(10 + j, 40, 10), T123)
                elif j == 5:
                    cast("v", j, slice(10 + j, 40, 10), T123)
                if j == 4:
                    # silu on ACT mid-stream (resid matmuls run ~then);
                    # one table-set switch, later Copy casts stay in-set
                    nc.scalar.activation(out=silu2[:], in_=xn[:],
                                         func=AF.Silu)
            cast("v", 6, slice(16, 40, 10), T123)
            cast("v", 7, slice(17, 40, 10), T123)
            cast("s", 8, slice(18, 40, 10), T123)
            cast("v", 9, slice(19, 40, 10), T123)

            # ---- spline matmuls: fp8 DoubleRow. t=0 block first (fed by
            # the quarter-granularity phase A), then the t=1..3 blocks
            # center-pair-major so each m-level's casts gate only 12
            # matmuls. Residual (bf16) slots in after the m=1 level, where
            # the PE would otherwise wait on mid-chain casts. ----
            ps = [psum.tile([128, O], f32, name=f"ps{bt}") for bt in range(4)]

            def spline_mm(t, m, bt, start, stop=False):
                s = t * C + 2 * m
                w, ws = wsp_slot(s)
                nc.tensor.matmul(
                    ps[bt], basis[:, s:s + 2, bt * 128:(bt + 1) * 128],
                    w[:, ws:ws + 2, :], start=start, stop=stop,
                    perf_mode=DR, skip_group_check=True)

            for m in range(C // 2):
                for bt in range(4):
                    spline_mm(0, m, bt, start=(m == 0))
            # keep the PE busy/hot through the t0->t123 cast wait
            for _ in range(6):
                nc.tensor.matmul(ps_warm[:], warm_w[:, :128], warm_w[:],
                                 start=True, stop=True)
            for m in range(C // 2 - 1):
                for t in (1, 2, 3):
                    for bt in range(4):
                        spline_mm(t, m, bt, start=False)
                if m == 2:
                    for bt in range(4):
                        for t in (0, 2):
                            nc.tensor.matmul(
                                ps[bt],
                                silu2[:, t:t + 2, bt * 128:(bt + 1) * 128],
                                wres_sb[:, t:t + 2, :], start=False,
                                stop=False, perf_mode=DR,
                                skip_group_check=True)
            # last m-level bank-major with staggered drain + out DMA
            out_sb = big.tile([128, NT, O], f32, tag="out")
            for bt in range(4):
                for t in (1, 2, 3):
                    spline_mm(t, C // 2 - 1, bt, start=False, stop=(t == 3))
                if bt == 3:
                    # split the last bank so its out-DMA starts sooner
                    nc.scalar.mul(out=out_sb[:, bt, 0:256],
                                  in_=ps[bt][:, 0:256], mul=1.0 / WSCALE)
                    nc.sync.dma_start(out=out_d[:, bt, 0:256],
                                      in_=out_sb[:, bt, 0:256])
                    nc.vector.tensor_scalar_mul(out_sb[:, bt, 256:512],
                                                ps[bt][:, 256:512],
                                                1.0 / WSCALE)
                    nc.sync.dma_start(out=out_d[:, bt, 256:512],
                                      in_=out_sb[:, bt, 256:512])
                    continue
                if bt % 2 == 0:
                    nc.scalar.mul(out=out_sb[:, bt, :], in_=ps[bt][:],
                                  mul=1.0 / WSCALE)
                else:
                    nc.vector.tensor_scalar_mul(out_sb[:, bt, :], ps[bt][:],
                                                1.0 / WSCALE)
                nc.sync.dma_start(out=out_d[:, bt, :], in_=out_sb[:, bt, :])
    nc.finalize()
    return nc


def _prep_inputs(x, coef, scale_base, scale_sp):
    """Host-side shard + layout prep (cheap numpy reshapes/casts)."""
    x = np.asarray(x, dtype=np.float32)
    coef = np.asarray(coef, dtype=np.float32)
    scale_base = np.asarray(scale_base, dtype=np.float32)
    scale_sp = np.asarray(scale_sp, dtype=np.float32)

    # spline weights: slot s = t*10 + j -> wsp[p, s, o] = wf[t*128+p, o, j]
    wf = coef * scale_sp.T[:, :, None] * WSCALE           # [I, O, C]
    wsp = wf.reshape(NT, 128, O, C).transpose(1, 0, 3, 2)  # [128, NT, C, O]
    wsp = np.ascontiguousarray(wsp.reshape(128, NSP, O)).astype(
        ml_dtypes.float8_e4m3)
    # residual weights carry the fp8 WSCALE (undone at PSUM drain)
    wres = (scale_base.T * WSCALE).reshape(NT, 128, O)
    wres = np.ascontiguousarray(wres.transpose(1, 0, 2)).astype(
        ml_dtypes.float8_e4m3)

    in_maps = []
    for k in range(N_CORES):
        xs = x[k * BS:(k + 1) * BS, :]                     # [BS, I]
        xt = np.ascontiguousarray(
            xs.T.reshape(NT, 128, BS).transpose(1, 0, 2))  # [128, NT, BS]
        in_maps.append({"xt": xt, "wsp": wsp, "wres": wres})
    return in_maps


def _run(in_maps, trace=False):
    if "antenv.axon_hooks" not in sys.modules:
        try:
            from trn_agent_boot.trn_boot import _ntff_profile_via_ctypes
            _hook = _ntff_profile_via_ctypes("/opt/axon/libaxon_pjrt.so")
            _mod = types.ModuleType("antenv.axon_hooks")
            _mod.get_axon_ntff_profile_hook = lambda: _hook
            sys.modules["antenv.axon_hooks"] = _mod
        except Exception:
            pass
    from concourse.bass_utils import run_bass_kernel_spmd

    if "nc" not in _CACHE:
        _CACHE["nc"] = _build()
    return run_bass_kernel_spmd(_CACHE["nc"], in_maps,
                                core_ids=list(range(N_CORES)), trace=trace)


def kernel(x, coef, scale_base, scale_sp):
    in_maps = _prep_inputs(x, coef, scale_base, scale_sp)
    res = _run(in_maps, trace=False)
    out = np.concatenate(
        [res.results[k]["out"].transpose(1, 0, 2).reshape(BS, O)
         for k in range(N_CORES)], axis=0)
    return out.astype(np.float32)
